# revision 1
# baseline (speedup 1.0000x reference)
"""TRN2 Bass kernel: 16-head MHA (B=2, S=2048, H=1024) sharded over 8 NeuronCores.

Sharding: data-parallel over batch (2) x tensor-parallel over head groups
(4 groups of 4 heads). Each core computes its 4 heads' attention for its batch
and a partial output projection; the host sums the 4 partials per batch,
transposes, and adds the output bias.

Per-core kernel (all activations transposed, bf16 on-chip, fp32 accumulation):
  qhT[d,q] = wq.T @ qT ; khT likewise ; vh[k,d] = (vT.T @ wv) with a ones
  column appended per head (rowsum trick).  Scores are computed transposed
  (s^T[k,q]), exp on ScalarE (scale=1/8 folded in), multiplicative {0,1} mask
  on VectorE, and the AV matmul accumulates x^T[d+1,q] in PSUM where row 64
  is the softmax denominator.  Normalization happens after: r = 1/rowsum on
  VectorE, broadcast across partitions via a K=1 matmul, multiplied into x.
"""

import sys

sys.path.insert(0, "/opt/trn_rl_repo")

from contextlib import ExitStack

import numpy as np
import ml_dtypes

import concourse.tile as tile
from concourse import bacc, mybir

BF16 = mybir.dt.bfloat16
F32 = mybir.dt.float32
F32R = mybir.dt.float32r
P = 128

_PROGRAM_CACHE = {}


def build_mha_program(S=2048, HID=1024, NH=4, DK=64, QB=1024, aug=False):
    """Build + compile the per-core SPMD Bass program."""
    D = NH * DK
    assert NH % 2 == 0 and DK == 64
    SH = S // P
    HT = HID // P
    HTa = HT + (1 if aug else 0)
    QBn = S // QB
    NS = min(512, QB)
    QH = QB // NS
    NQ = S // NS
    DC = D // P
    NPAIR = NH // 2
    GW = DK + 2                  # 64 data cols + rowsum-ones col + pad (4B-aligned groups)

    nc = bacc.Bacc("TRN2", target_bir_lowering=False, debug=False)

    qT_d = nc.dram_tensor("qT", [HTa * P, S], BF16, kind="ExternalInput").ap()
    kT_d = nc.dram_tensor("kT", [HTa * P, S], BF16, kind="ExternalInput").ap()
    vT_d = nc.dram_tensor("vT", [HTa * P, S], BF16, kind="ExternalInput").ap()
    maskT_d = nc.dram_tensor("maskT", [S, S], BF16, kind="ExternalInput").ap()
    wq_d = nc.dram_tensor("wq", [HTa * P, D], BF16, kind="ExternalInput").ap()
    wk_d = nc.dram_tensor("wk", [HTa * P, D], BF16, kind="ExternalInput").ap()
    wv_d = nc.dram_tensor("wv", [HTa * P, D], BF16, kind="ExternalInput").ap()
    wo_d = nc.dram_tensor("wo", [D, HID], BF16, kind="ExternalInput").ap()
    y_d = nc.dram_tensor("y", [HID, S], F32, kind="ExternalOutput").ap()
    # DRAM bounce buffer for partition-broadcasting the softmax reciprocals
    rb_d = nc.dram_tensor("r_bounce", [32 * QBn, QB], F32).ap()

    Exp = mybir.ActivationFunctionType.Exp

    with tile.TileContext(nc) as tc:
        with ExitStack() as ctx:
            persist = ctx.enter_context(tc.tile_pool(name="persist", bufs=1))
            qh_t = [persist.tile([P, S], BF16, tag=f"qh{d}", name=f"qh{d}")
                    for d in range(DC)]
            kh_t = [persist.tile([P, S], BF16, tag=f"kh{d}", name=f"kh{d}")
                    for d in range(DC)]
            vh_t = [persist.tile([P, NH * GW], BF16, tag=f"vh{s}", name=f"vh{s}")
                    for s in range(SH)]
            xu_t = [persist.tile([P, S], BF16, tag=f"xu{p}", name=f"xu{p}")
                    for p in range(NPAIR)]
            xn_t = [persist.tile([P, S], BF16, tag=f"xn{p}", name=f"xn{p}")
                    for p in range(NPAIR)]
            wo_t = [persist.tile([P, HID], BF16, tag=f"wo{p}", name=f"wo{p}")
                    for p in range(NPAIR)]
            rs_t = persist.tile([32 * QBn, QB], F32, tag="rs", name="rs")
            r_t = persist.tile([32 * QBn, QB], F32, tag="r", name="r")

            for pr in range(NPAIR):
                nc.sync.dma_start(wo_t[pr][:], wo_d[pr * P:(pr + 1) * P, :])

            # phase 1: projections
            with ExitStack() as ph1:
                inp = ph1.enter_context(tc.tile_pool(name="inp", bufs=1))
                wp = ph1.enter_context(tc.tile_pool(name="wp", bufs=1))
                ps1 = ph1.enter_context(
                    tc.tile_pool(name="ps1", bufs=1, space="PSUM"))

                qT_t = [inp.tile([P, S], BF16, tag=f"qT{i}", name=f"qT{i}")
                        for i in range(HTa)]
                kT_t = [inp.tile([P, S], BF16, tag=f"kT{i}", name=f"kT{i}")
                        for i in range(HTa)]
                vT_t = [inp.tile([P, S], BF16, tag=f"vT{i}", name=f"vT{i}")
                        for i in range(HTa)]
                wq_t = [wp.tile([P, D], BF16, tag=f"wq{i}", name=f"wq{i}")
                        for i in range(HTa)]
                wk_t = [wp.tile([P, D], BF16, tag=f"wk{i}", name=f"wk{i}")
                        for i in range(HTa)]
                wv_t = [wp.tile([P, D], BF16, tag=f"wv{i}", name=f"wv{i}")
                        for i in range(HTa)]
                for i in range(HTa):
                    sl = slice(i * P, (i + 1) * P)
                    nc.sync.dma_start(qT_t[i][:], qT_d[sl, :])
                    nc.sync.dma_start(kT_t[i][:], kT_d[sl, :])
                    nc.sync.dma_start(vT_t[i][:], vT_d[sl, :])
                    nc.sync.dma_start(wq_t[i][:], wq_d[sl, :])
                    nc.sync.dma_start(wk_t[i][:], wk_d[sl, :])
                    nc.sync.dma_start(wv_t[i][:], wv_d[sl, :])

                for (src_t, w_t, dst) in ((qT_t, wq_t, qh_t),
                                          (kT_t, wk_t, kh_t)):
                    for dc in range(DC):
                        psl = [ps1.tile([P, NS], F32, tag=f"p1_{qc}",
                                        name=f"p1_{qc}") for qc in range(NQ)]
                        for i in range(HTa):
                            for qc in range(NQ):
                                nc.tensor.matmul(
                                    psl[qc][:],
                                    w_t[i][:, dc * P:(dc + 1) * P],
                                    src_t[i][:, qc * NS:(qc + 1) * NS],
                                    start=(i == 0), stop=(i == HTa - 1))
                        for qc in range(NQ):
                            nc.vector.tensor_copy(
                                dst[dc][:, qc * NS:(qc + 1) * NS], psl[qc][:])

                # vh[k, d]: direct projection, ones cols (rowsum trick) from
                # the memset survive the grouped copy
                for sc in range(SH):
                    ps_v = ps1.tile([P, D], F32, tag="p1v", name="p1v", bufs=2)
                    for i in range(HTa):
                        nc.tensor.matmul(
                            ps_v[:],
                            vT_t[i][:, sc * P:(sc + 1) * P],
                            wv_t[i][:],
                            start=(i == 0), stop=(i == HTa - 1))
                    nc.vector.memset(vh_t[sc][:], 1.0)
                    dst_v = vh_t[sc][:].rearrange(
                        "p (h c) -> p h c", c=GW)[:, :, 0:DK]
                    src_v = ps_v[:].rearrange("p (h c) -> p h c", c=DK)
                    nc.vector.tensor_copy(dst_v, src_v)

            # phase 2+3+4 fused: attention per (q-block, head); after each
            # q-block completes, its normalize + output projection are
            # emitted interleaved into the NEXT q-block's attention as PE
            # filler work (the attention pipeline is ScalarE-bound).
            with ExitStack() as ph2:
                mp = ph2.enter_context(tc.tile_pool(name="mask", bufs=1))
                pp = ph2.enter_context(tc.tile_pool(name="pexp", bufs=3))
                pmp = ph2.enter_context(tc.tile_pool(name="pmask", bufs=8))
                stg = ph2.enter_context(tc.tile_pool(name="stg", bufs=2))
                rbp = ph2.enter_context(tc.tile_pool(name="rbp", bufs=3))
                ysb = ph2.enter_context(tc.tile_pool(name="ysb", bufs=4))
                sps = ph2.enter_context(
                    tc.tile_pool(name="sps", bufs=3, space="PSUM"))
                xps = ph2.enter_context(
                    tc.tile_pool(name="xps", bufs=1, space="PSUM"))

                mask_t = [mp.tile([P, S], BF16, tag=f"m{i}", name=f"m{i}")
                          for i in range(SH)]
                for i in range(SH):
                    nc.scalar.dma_start(mask_t[i][:],
                                        maskT_d[i * P:(i + 1) * P, :])

                def attention_head(qb, h):
                    """QK -> exp -> mask -> AV (deep-lagged) for one head.

                    AV matmuls are emitted LAG chunks late so their semaphore
                    waits never block the PE (a blocking PE wait starts a
                    vicious cycle: PE idles, HAM drops it to 1.2 GHz, the
                    slowed pipeline then blocks on every wait)."""
                    LAG = 6
                    qsl = slice(qb * QB, (qb + 1) * QB)
                    ht, hb = divmod(h, 2)
                    hsl = slice(64 * hb, 64 * hb + 64)
                    x_ps = xps.tile([P, QB], F32, tag="x", name="x")
                    pending = []

                    def emit_av(kc, pm_t):
                        for qh_ in range(QH):
                            nsl = slice(qh_ * NS, (qh_ + 1) * NS)
                            nc.tensor.matmul(
                                x_ps[:DK + 1, nsl],
                                vh_t[kc][:, h * GW:h * GW + DK + 1],
                                pm_t[:, nsl],
                                start=(kc == 0), stop=(kc == SH - 1),
                                skip_group_check=True)

                    for kc in range(SH):
                        s_ps = sps.tile([P, QB], F32, tag="s", name="s")
                        for qh_ in range(QH):
                            nsl = slice(qh_ * NS, (qh_ + 1) * NS)
                            nc.tensor.matmul(
                                s_ps[:, nsl],
                                kh_t[ht][hsl, kc * P:(kc + 1) * P],
                                qh_t[ht][hsl, qb * QB + qh_ * NS:
                                         qb * QB + (qh_ + 1) * NS],
                                start=True, stop=True)
                        p_t = pp.tile([P, QB], BF16, tag="p", name="p")
                        nc.scalar.activation(p_t[:], s_ps[:], Exp, scale=0.125)
                        pm_t = pmp.tile([P, QB], BF16, tag="pm", name="pm")
                        nc.vector.tensor_mul(
                            pm_t[:], p_t[:], mask_t[kc][:, qsl])
                        pending.append((kc, pm_t))
                        if len(pending) > LAG:
                            emit_av(*pending.pop(0))
                    for item in pending:
                        emit_av(*item)

                    row = qb * 32 + h
                    stage = stg.tile([GW, QB], F32, tag="stg", name="stg")
                    nc.vector.tensor_copy(stage[DK:DK + 1, :],
                                          x_ps[DK:DK + 1, :])
                    nc.sync.dma_start(rs_t[row:row + 1, :],
                                      stage[DK:DK + 1, :])
                    nc.vector.tensor_copy(xu_t[ht][hsl, qsl], x_ps[:DK, :])

                def normalize_qblock(qb):
                    """reciprocal of this q-block's rowsums, partition
                    broadcast via DRAM round-trip, then xn = xu * r."""
                    qsl = slice(qb * QB, (qb + 1) * QB)
                    rows = slice(qb * 32, qb * 32 + NH)
                    nc.vector.tensor_scalar_max(rs_t[rows, :], rs_t[rows, :],
                                                1e-30)
                    nc.vector.reciprocal(r_t[rows, :], rs_t[rows, :])
                    nc.sync.dma_start(rb_d[rows, :], r_t[rows, :])
                    for pr in range(NPAIR):
                        rb = rbp.tile([P, QB], F32, tag="rb", name="rb")
                        for hb in range(2):
                            row = qb * 32 + 2 * pr + hb
                            eng = (nc.sync, nc.scalar)[(pr + hb) % 2]
                            eng.dma_start(
                                rb[64 * hb:64 * hb + 64, :],
                                rb_d[row:row + 1, :].broadcast_to([64, QB]))
                        nc.vector.tensor_mul(
                            xn_t[pr][:, qsl], xu_t[pr][:, qsl], rb[:])

                def oproj_chunk(qb, hc):
                    """y[hc, qb] = sum over pairs wo^T @ xn, via an s-pool
                    PSUM slot; runs as PE filler inside later q-blocks."""
                    for qh_ in range(QH):
                        y_ps = sps.tile([P, NS], F32, tag="s", name="yps")
                        qc0 = qb * QH + qh_
                        for pr in range(NPAIR):
                            nc.tensor.matmul(
                                y_ps[:],
                                wo_t[pr][:, hc * P:(hc + 1) * P],
                                xn_t[pr][:, qc0 * NS:(qc0 + 1) * NS],
                                start=(pr == 0), stop=(pr == NPAIR - 1))
                        y_sb = ysb.tile([P, NS], F32, tag="ysb", name="ysb")
                        nc.vector.tensor_copy(y_sb[:], y_ps[:])
                        (nc.sync, nc.scalar, nc.gpsimd)[qc0 % 3].dma_start(
                            y_d[hc * P:(hc + 1) * P,
                                qc0 * NS:(qc0 + 1) * NS],
                            y_sb[:])

                hc_groups = [[hc for hc in range(HT) if hc % NH == h]
                             for h in range(NH)]
                for qb in range(QBn):
                    for h in range(NH):
                        attention_head(qb, h)
                        if qb >= 1:
                            for hc in hc_groups[h]:
                                oproj_chunk(qb - 1, hc)
                    normalize_qblock(qb)
                for hc in range(HT):
                    oproj_chunk(QBn - 1, hc)

    nc.compile()
    return nc


def make_in_maps(q, k, v, mask, Wq, bq, Wk, bk, Wv, bv, Wo,
                 n_cores=8, NH=4, DK=64, aug=False):
    bf = ml_dtypes.bfloat16
    B, S, HID = q.shape
    D = NH * DK
    n_hg = n_cores // B

    def with_aug(xT, bias_row):
        pad = np.zeros((P, xT.shape[1]), xT.dtype)
        pad[0, :] = bias_row
        return np.concatenate([xT, pad], axis=0)

    per_batch = {}
    for b in range(B):
        qT = np.ascontiguousarray(q[b].T).astype(bf)
        kT = np.ascontiguousarray(k[b].T).astype(bf)
        vT = np.ascontiguousarray(v[b].T).astype(bf)
        if aug:
            one = np.ones((S,), np.float32).astype(bf)
            qT, kT, vT = with_aug(qT, one), with_aug(kT, one), with_aug(vT, one)
        per_batch[b] = (qT, kT, vT,
                        np.ascontiguousarray(mask[b, 0].T != 0).astype(bf))

    in_maps = []
    for core in range(n_cores):
        b, hg = divmod(core, n_hg)
        hsl = slice(hg * D, (hg + 1) * D)
        wq = Wq[:, hsl].astype(bf)
        wk = Wk[:, hsl].astype(bf)
        wv = Wv[:, hsl].astype(bf)
        if aug:
            wq = with_aug(wq, bq[hsl].astype(bf))
            wk = with_aug(wk, bk[hsl].astype(bf))
            wv = with_aug(wv, bv[hsl].astype(bf))
        qT, kT, vT, mT = per_batch[b]
        in_maps.append(dict(
            qT=qT, kT=kT, vT=vT, maskT=mT,
            wq=np.ascontiguousarray(wq), wk=np.ascontiguousarray(wk),
            wv=np.ascontiguousarray(wv),
            wo=np.ascontiguousarray(Wo[hsl, :]).astype(bf),
        ))
    return in_maps


def combine_outputs(results, B, S, HID, bo, n_cores=8):
    n_hg = n_cores // B
    out = np.zeros((B, S, HID), np.float32)
    for core in range(n_cores):
        b = core // n_hg
        out[b] += results[core]["y"].T
    return out + bo.astype(np.float32)


def run_mha(q, k, v, mask, Wq, bq, Wk, bk, Wv, bv, Wo, bo, trace=False):
    from concourse.bass_utils import run_bass_kernel_spmd

    B, S, HID = q.shape
    n_cores = 8
    aug = bool(np.any(bq) or np.any(bk) or np.any(bv))
    key = (S, HID, aug)
    if key not in _PROGRAM_CACHE:
        _PROGRAM_CACHE[key] = build_mha_program(S=S, HID=HID, aug=aug)
    nc = _PROGRAM_CACHE[key]
    in_maps = make_in_maps(q, k, v, mask, Wq, bq, Wk, bk, Wv, bv, Wo,
                           n_cores=n_cores, aug=aug)
    res = run_bass_kernel_spmd(nc, in_maps, list(range(n_cores)), trace=trace)
    out = combine_outputs(res.results, B, S, HID, bo, n_cores=n_cores)
    return out, res


def kernel(q, k, v, mask, Wq, bq, Wk, bk, Wv, bv, Wo, bo):
    q = np.asarray(q, np.float32)
    k = np.asarray(k, np.float32)
    v = np.asarray(v, np.float32)
    mask = np.asarray(mask)
    out, _ = run_mha(q, k, v, mask,
                     np.asarray(Wq, np.float32), np.asarray(bq, np.float32),
                     np.asarray(Wk, np.float32), np.asarray(bk, np.float32),
                     np.asarray(Wv, np.float32), np.asarray(bv, np.float32),
                     np.asarray(Wo, np.float32), np.asarray(bo, np.float32))
    return out



# revision 11
# speedup vs baseline: 1.1623x; 1.1623x over previous
"""TRN2 Bass kernel: 16-head MHA (B=2, S=2048, H=1024) sharded over 8 NeuronCores.

Sharding: data-parallel over batch (2) x tensor-parallel over head groups
(4 groups of 4 heads). Each core computes its 4 heads' attention for its batch
and a partial output projection; the host sums the 4 partials per batch,
transposes, and adds the output bias.

Per-core kernel (all activations transposed, bf16 on-chip, fp32 accumulation):
  qhT[d,q] = wq.T @ qT ; khT likewise ; vh[k,d] = (vT.T @ wv) with a ones
  column appended per head (rowsum trick).  Scores are computed transposed
  (s^T[k,q]), exp on ScalarE (scale=1/8 folded in), multiplicative {0,1} mask
  on VectorE (some chunks offloaded to GpSimd), and the AV matmul accumulates
  x^T[d+1,q] in PSUM where row 64 is the softmax denominator.

Normalization per (qb, head): one VectorE cast drains x^T plus its rowsum row
to a bf16 stage tile; small SBUF->SBUF DMAs gather the 4 rowsum rows onto
adjacent partitions; reciprocal_approx_fast + casts on VectorE; GpSimd
partition_broadcast fans r out across the 64 d-partitions; one bf16 multiply
produces xn.  No DRAM bounce, nothing on the ScalarE queue (EXPs never block).

DMA issue load is spread across the Sync/GpSimd/Scalar/Vector queues in
data-need order so the first projection matmul starts ~4us in, and mask tiles
stream on the GpSimd queue behind the projection inputs.
"""

import sys

sys.path.insert(0, "/opt/trn_rl_repo")

from contextlib import ExitStack

import numpy as np
import ml_dtypes

import concourse.tile as tile
from concourse import bacc, mybir

BF16 = mybir.dt.bfloat16
F32 = mybir.dt.float32
P = 128

_PROGRAM_CACHE = {}


def build_mha_program(S=2048, HID=1024, NH=4, DK=64, QB=1024, aug=False):
    """Build + compile the per-core SPMD Bass program."""
    D = NH * DK
    assert NH % 2 == 0 and DK == 64
    SH = S // P
    HT = HID // P
    HTa = HT + (1 if aug else 0)
    QBn = S // QB
    NS = min(512, QB)
    QH = QB // NS
    NQ = S // NS
    DC = D // P
    NPAIR = NH // 2
    GW = DK + 2                  # 64 data cols + rowsum-ones col + pad

    nc = bacc.Bacc("TRN2", target_bir_lowering=False, debug=False)

    qT_d = nc.dram_tensor("qT", [HTa * P, S], BF16, kind="ExternalInput").ap()
    kT_d = nc.dram_tensor("kT", [HTa * P, S], BF16, kind="ExternalInput").ap()
    vT_d = nc.dram_tensor("vT", [HTa * P, S], BF16, kind="ExternalInput").ap()
    maskT_d = nc.dram_tensor("maskT", [S, S], BF16, kind="ExternalInput").ap()
    wq_d = nc.dram_tensor("wq", [HTa * P, D], BF16, kind="ExternalInput").ap()
    wk_d = nc.dram_tensor("wk", [HTa * P, D], BF16, kind="ExternalInput").ap()
    wv_d = nc.dram_tensor("wv", [HTa * P, D], BF16, kind="ExternalInput").ap()
    wo_d = nc.dram_tensor("wo", [D, HID], BF16, kind="ExternalInput").ap()
    y_d = nc.dram_tensor("y", [HID, S], F32, kind="ExternalOutput").ap()
    # DRAM bounce for the softmax-reciprocal partition broadcast (SBUF DMA
    # sources cannot have zero partition stride; DRAM reads can)
    rb_d = nc.dram_tensor("r_bounce", [NH * QBn, QB], BF16).ap()

    Exp = mybir.ActivationFunctionType.Exp

    with tile.TileContext(nc) as tc:
        with ExitStack() as ctx:
            persist = ctx.enter_context(tc.tile_pool(name="persist", bufs=1))
            qh_t = [persist.tile([P, S], BF16, tag=f"qh{d}", name=f"qh{d}")
                    for d in range(DC)]
            kh_t = [persist.tile([P, S], BF16, tag=f"kh{d}", name=f"kh{d}")
                    for d in range(DC)]
            vh_t = [persist.tile([P, NH * GW], BF16, tag=f"vh{s}", name=f"vh{s}")
                    for s in range(SH)]
            # per-head unnormalized x^T (rows 0..63) + rowsum (row 64), bf16
            stage_t = [persist.tile([DK + 1, S], BF16, tag=f"st{h}",
                                    name=f"st{h}") for h in range(NH)]
            xn_t = [persist.tile([P, S], BF16, tag=f"xn{p}", name=f"xn{p}")
                    for p in range(NPAIR)]
            wo_t = [persist.tile([P, HID], BF16, tag=f"wo{p}", name=f"wo{p}")
                    for p in range(NPAIR)]
            # rowsum gather rows + reciprocal scratch
            rs4 = persist.tile([NH, QB], BF16, tag="rs4", name="rs4")
            rsf = persist.tile([NH, QB], F32, tag="rsf", name="rsf")
            rff = persist.tile([NH, QB], F32, tag="rff", name="rff")
            r4b = persist.tile([NH, QB], BF16, tag="r4b", name="r4b")
            rb_t = [persist.tile([DK, QB], BF16, tag=f"rb{h}", name=f"rb{h}")
                    for h in range(NH)]

            for pr in range(NPAIR):
                nc.scalar.dma_start(wo_t[pr][:], wo_d[pr * P:(pr + 1) * P, :])

            # phase 1: projections
            with ExitStack() as ph1:
                inp = ph1.enter_context(tc.tile_pool(name="inp", bufs=1))
                wp = ph1.enter_context(tc.tile_pool(name="wp", bufs=1))
                ps1 = ph1.enter_context(
                    tc.tile_pool(name="ps1", bufs=1, space="PSUM"))

                qT_t = [inp.tile([P, S], BF16, tag=f"qT{i}", name=f"qT{i}")
                        for i in range(HTa)]
                kT_t = [inp.tile([P, S], BF16, tag=f"kT{i}", name=f"kT{i}")
                        for i in range(HTa)]
                vT_t = [inp.tile([P, S], BF16, tag=f"vT{i}", name=f"vT{i}")
                        for i in range(HTa)]
                wq_t = [wp.tile([P, D], BF16, tag=f"wq{i}", name=f"wq{i}")
                        for i in range(HTa)]
                wk_t = [wp.tile([P, D], BF16, tag=f"wk{i}", name=f"wk{i}")
                        for i in range(HTa)]
                wv_t = [wp.tile([P, D], BF16, tag=f"wv{i}", name=f"wv{i}")
                        for i in range(HTa)]
                # DMA issue cost is ~650-780ns serial per issuing engine, so
                # spread the input loads across four queues in need-order.
                for i in range(HTa):
                    sl = slice(i * P, (i + 1) * P)
                    nc.sync.dma_start(qT_t[i][:], qT_d[sl, :])
                    nc.sync.dma_start(wq_t[i][:], wq_d[sl, :])
                    nc.gpsimd.dma_start(kT_t[i][:], kT_d[sl, :])
                    nc.gpsimd.dma_start(wk_t[i][:], wk_d[sl, :])
                    nc.scalar.dma_start(vT_t[i][:], vT_d[sl, :])
                    nc.scalar.dma_start(wv_t[i][:], wv_d[sl, :])

                for (src_t, w_t, dst) in ((qT_t, wq_t, qh_t),
                                          (kT_t, wk_t, kh_t)):
                    for dc in range(DC):
                        psl = [ps1.tile([P, NS], F32, tag=f"p1_{qc}",
                                        name=f"p1_{qc}") for qc in range(NQ)]
                        for i in range(HTa):
                            for qc in range(NQ):
                                nc.tensor.matmul(
                                    psl[qc][:],
                                    w_t[i][:, dc * P:(dc + 1) * P],
                                    src_t[i][:, qc * NS:(qc + 1) * NS],
                                    start=(i == 0), stop=(i == HTa - 1))
                        for qc in range(NQ):
                            nc.vector.tensor_copy(
                                dst[dc][:, qc * NS:(qc + 1) * NS], psl[qc][:])

                # vh[k, d]: direct projection, ones cols (rowsum trick) from
                # the memset survive the grouped copy
                for sc in range(SH):
                    ps_v = ps1.tile([P, D], F32, tag="p1v", name="p1v", bufs=2)
                    for i in range(HTa):
                        nc.tensor.matmul(
                            ps_v[:],
                            vT_t[i][:, sc * P:(sc + 1) * P],
                            wv_t[i][:],
                            start=(i == 0), stop=(i == HTa - 1))
                    nc.vector.memset(vh_t[sc][:], 1.0)
                    dst_v = vh_t[sc][:].rearrange(
                        "p (h c) -> p h c", c=GW)[:, :, 0:DK]
                    src_v = ps_v[:].rearrange("p (h c) -> p h c", c=DK)
                    nc.vector.tensor_copy(dst_v, src_v)

            # phase 2+3+4 fused: attention per (q-block, head); after each
            # q-block completes, its normalize + output projection are
            # emitted interleaved into the NEXT q-block's attention as PE
            # filler work.
            with ExitStack() as ph2:
                mp = ph2.enter_context(tc.tile_pool(name="mask", bufs=1))
                pp = ph2.enter_context(tc.tile_pool(name="pexp", bufs=3))
                pmp = ph2.enter_context(tc.tile_pool(name="pmask", bufs=8))
                ysb = ph2.enter_context(tc.tile_pool(name="ysb", bufs=4))
                sps = ph2.enter_context(
                    tc.tile_pool(name="sps", bufs=3, space="PSUM"))
                xps = ph2.enter_context(
                    tc.tile_pool(name="xps", bufs=1, space="PSUM"))

                # mask tiles alias the freed phase-1 input space; the WAR
                # semaphores gate each DMA on its region freeing, and the
                # gpsimd queue (not scalar) carries the issues so EXPs on
                # the scalar queue never wait behind them.
                mask_t = [mp.tile([P, S], BF16, tag=f"m{i}", name=f"m{i}")
                          for i in range(SH)]
                for i in range(SH):
                    nc.gpsimd.dma_start(mask_t[i][:],
                                        maskT_d[i * P:(i + 1) * P, :])

                def attention_head(qb, h):
                    """QK -> exp -> mask -> AV (deep-lagged) for one head.

                    AV matmuls are emitted LAG chunks late so their semaphore
                    waits never block the PE."""
                    LAG = 6
                    qsl = slice(qb * QB, (qb + 1) * QB)
                    ht, hb = divmod(h, 2)
                    hsl = slice(64 * hb, 64 * hb + 64)
                    x_ps = xps.tile([P, QB], F32, tag="x", name="x")
                    pending = []

                    def emit_av(kc, pm_t):
                        for qh_ in range(QH):
                            nsl = slice(qh_ * NS, (qh_ + 1) * NS)
                            nc.tensor.matmul(
                                x_ps[:DK + 1, nsl],
                                vh_t[kc][:, h * GW:h * GW + DK + 1],
                                pm_t[:, nsl],
                                start=(kc == 0), stop=(kc == SH - 1),
                                skip_group_check=True)

                    for kc in range(SH):
                        s_ps = sps.tile([P, QB], F32, tag="s", name="s")
                        for qh_ in range(QH):
                            nsl = slice(qh_ * NS, (qh_ + 1) * NS)
                            nc.tensor.matmul(
                                s_ps[:, nsl],
                                kh_t[ht][hsl, kc * P:(kc + 1) * P],
                                qh_t[ht][hsl, qb * QB + qh_ * NS:
                                         qb * QB + (qh_ + 1) * NS],
                                start=True, stop=True)
                        p_t = pp.tile([P, QB], BF16, tag="p", name="p")
                        nc.scalar.activation(p_t[:], s_ps[:], Exp, scale=0.125)
                        pm_t = pmp.tile([P, QB], BF16, tag="pm", name="pm")
                        # offload some mask-muls to the idle GpSimd engine,
                        # but keep the tail head all-DVE (GpSimd is slower
                        # per-op and the tail has no slack).
                        off = (kc % 6 == 5) and not (qb == QBn - 1 and
                                                     h == NH - 1)
                        eng = nc.gpsimd if off else nc.vector
                        eng.tensor_mul(pm_t[:], p_t[:], mask_t[kc][:, qsl])
                        pending.append((kc, pm_t))
                        if len(pending) > LAG:
                            emit_av(*pending.pop(0))
                    for item in pending:
                        emit_av(*item)

                    # single cast drains x rows AND the rowsum row to bf16
                    nc.vector.tensor_copy(stage_t[h][:, qsl],
                                          x_ps[:DK + 1, :])

                def normalize_qblock(qb):
                    """r = 1/rowsum per (head, q); broadcast across the 64
                    d-partitions on GpSimd; xn = x * r on VectorE."""
                    qsl = slice(qb * QB, (qb + 1) * QB)
                    for h in range(NH):
                        nc.sync.dma_start(rs4[h:h + 1, :],
                                          stage_t[h][DK:DK + 1, qsl])
                    nc.vector.tensor_copy(rsf[:], rs4[:])
                    nc.vector.reciprocal_approx_fast(rff[:], rsf[:])
                    nc.vector.tensor_copy(r4b[:], rff[:])
                    rows = slice(qb * NH, qb * NH + NH)
                    nc.sync.dma_start(rb_d[rows, :], r4b[:])
                    for h in range(NH):
                        eng = (nc.sync, nc.gpsimd)[h % 2]
                        eng.dma_start(
                            rb_t[h][:],
                            rb_d[qb * NH + h:qb * NH + h + 1,
                                 :].broadcast_to([DK, QB]))
                    for h in range(NH):
                        ht, hb = divmod(h, 2)
                        hsl = slice(64 * hb, 64 * hb + 64)
                        nc.vector.tensor_mul(xn_t[ht][hsl, qsl],
                                             stage_t[h][:DK, qsl],
                                             rb_t[h][:])

                def oproj_chunk(qb, hc):
                    """y[hc, qb] = sum over pairs wo^T @ xn, via an s-pool
                    PSUM slot; runs as PE filler inside later q-blocks."""
                    for qh_ in range(QH):
                        y_ps = sps.tile([P, NS], F32, tag="s", name="yps")
                        qc0 = qb * QH + qh_
                        for pr in range(NPAIR):
                            nc.tensor.matmul(
                                y_ps[:],
                                wo_t[pr][:, hc * P:(hc + 1) * P],
                                xn_t[pr][:, qc0 * NS:(qc0 + 1) * NS],
                                start=(pr == 0), stop=(pr == NPAIR - 1))
                        y_sb = ysb.tile([P, NS], F32, tag="ysb", name="ysb")
                        nc.vector.tensor_copy(y_sb[:], y_ps[:])
                        (nc.sync, nc.gpsimd)[qc0 % 2].dma_start(
                            y_d[hc * P:(hc + 1) * P,
                                qc0 * NS:(qc0 + 1) * NS],
                            y_sb[:])

                hc_groups = [[hc for hc in range(HT) if hc % NH == h]
                             for h in range(NH)]
                for qb in range(QBn):
                    for h in range(NH):
                        attention_head(qb, h)
                        if qb >= 1:
                            for hc in hc_groups[h]:
                                oproj_chunk(qb - 1, hc)
                    normalize_qblock(qb)
                for hc in range(HT):
                    oproj_chunk(QBn - 1, hc)

    nc.compile()
    return nc


def make_in_maps(q, k, v, mask, Wq, bq, Wk, bk, Wv, bv, Wo,
                 n_cores=8, NH=4, DK=64, aug=False):
    bf = ml_dtypes.bfloat16
    B, S, HID = q.shape
    D = NH * DK
    n_hg = n_cores // B

    def with_aug(xT, bias_row):
        pad = np.zeros((P, xT.shape[1]), xT.dtype)
        pad[0, :] = bias_row
        return np.concatenate([xT, pad], axis=0)

    per_batch = {}
    for b in range(B):
        qT = np.ascontiguousarray(q[b].T).astype(bf)
        kT = np.ascontiguousarray(k[b].T).astype(bf)
        vT = np.ascontiguousarray(v[b].T).astype(bf)
        if aug:
            one = np.ones((S,), np.float32).astype(bf)
            qT, kT, vT = with_aug(qT, one), with_aug(kT, one), with_aug(vT, one)
        per_batch[b] = (qT, kT, vT,
                        np.ascontiguousarray(mask[b, 0].T != 0).astype(bf))

    in_maps = []
    for core in range(n_cores):
        b, hg = divmod(core, n_hg)
        hsl = slice(hg * D, (hg + 1) * D)
        wq = Wq[:, hsl].astype(bf)
        wk = Wk[:, hsl].astype(bf)
        wv = Wv[:, hsl].astype(bf)
        if aug:
            wq = with_aug(wq, bq[hsl].astype(bf))
            wk = with_aug(wk, bk[hsl].astype(bf))
            wv = with_aug(wv, bv[hsl].astype(bf))
        qT, kT, vT, mT = per_batch[b]
        in_maps.append(dict(
            qT=qT, kT=kT, vT=vT, maskT=mT,
            wq=np.ascontiguousarray(wq), wk=np.ascontiguousarray(wk),
            wv=np.ascontiguousarray(wv),
            wo=np.ascontiguousarray(Wo[hsl, :]).astype(bf),
        ))
    return in_maps


def combine_outputs(results, B, S, HID, bo, n_cores=8):
    n_hg = n_cores // B
    out = np.zeros((B, S, HID), np.float32)
    for core in range(n_cores):
        b = core // n_hg
        out[b] += results[core]["y"].T
    return out + bo.astype(np.float32)


def run_mha(q, k, v, mask, Wq, bq, Wk, bk, Wv, bv, Wo, bo, trace=False):
    from concourse.bass_utils import run_bass_kernel_spmd

    B, S, HID = q.shape
    n_cores = 8
    aug = bool(np.any(bq) or np.any(bk) or np.any(bv))
    key = (S, HID, aug)
    if key not in _PROGRAM_CACHE:
        _PROGRAM_CACHE[key] = build_mha_program(S=S, HID=HID, aug=aug)
    nc = _PROGRAM_CACHE[key]
    in_maps = make_in_maps(q, k, v, mask, Wq, bq, Wk, bk, Wv, bv, Wo,
                           n_cores=n_cores, aug=aug)
    res = run_bass_kernel_spmd(nc, in_maps, list(range(n_cores)), trace=trace)
    out = combine_outputs(res.results, B, S, HID, bo, n_cores=n_cores)
    return out, res


def kernel(q, k, v, mask, Wq, bq, Wk, bk, Wv, bv, Wo, bo):
    q = np.asarray(q, np.float32)
    k = np.asarray(k, np.float32)
    v = np.asarray(v, np.float32)
    mask = np.asarray(mask)
    out, _ = run_mha(q, k, v, mask,
                     np.asarray(Wq, np.float32), np.asarray(bq, np.float32),
                     np.asarray(Wk, np.float32), np.asarray(bk, np.float32),
                     np.asarray(Wv, np.float32), np.asarray(bv, np.float32),
                     np.asarray(Wo, np.float32), np.asarray(bo, np.float32))
    return out


# revision 12
# speedup vs baseline: 1.2794x; 1.1007x over previous
"""TRN2 Bass kernel: 16-head MHA (B=2, S=2048, H=1024) sharded over 8 NeuronCores.

Sharding: data-parallel over batch (2) x tensor-parallel over head groups
(4 groups of 4 heads). Each core computes its 4 heads' attention for its batch
and a partial output projection; the host sums the 4 partials per batch,
transposes, and adds the output bias.

Per-core kernel (all activations transposed, bf16 on-chip, fp32 accumulation):
  qhT[d,q] = wq.T @ qT ; khT likewise ; vh[k,d] = (vT.T @ wv) with a ones
  column appended per head (rowsum trick).  Scores are computed transposed
  (s^T[k,q]), exp on ScalarE (scale=1/8 folded in), multiplicative {0,1} mask
  on VectorE (some chunks offloaded to GpSimd), and the AV matmul accumulates
  x^T[d+1,q] in PSUM where row 64 is the softmax denominator.

Normalization per (qb, head): one VectorE cast drains x^T plus its rowsum row
to a bf16 stage tile; small SBUF->SBUF DMAs gather the 4 rowsum rows onto
adjacent partitions; reciprocal_approx_fast + casts on VectorE; GpSimd
partition_broadcast fans r out across the 64 d-partitions; one bf16 multiply
produces xn.  No DRAM bounce, nothing on the ScalarE queue (EXPs never block).

DMA issue load is spread across the Sync/GpSimd/Scalar/Vector queues in
data-need order so the first projection matmul starts ~4us in, and mask tiles
stream on the GpSimd queue behind the projection inputs.
"""

import sys

sys.path.insert(0, "/opt/trn_rl_repo")

from contextlib import ExitStack

import numpy as np
import ml_dtypes

import concourse.tile as tile
from concourse import bacc, mybir

BF16 = mybir.dt.bfloat16
F32 = mybir.dt.float32
P = 128

_PROGRAM_CACHE = {}


def build_mha_program(S=2048, HID=1024, NH=4, DK=64, QB=1024, aug=False):
    """Build + compile the per-core SPMD Bass program."""
    D = NH * DK
    assert NH % 2 == 0 and DK == 64
    SH = S // P
    HT = HID // P
    HTa = HT + (1 if aug else 0)
    QBn = S // QB
    NS = min(512, QB)
    QH = QB // NS
    NQ = S // NS
    DC = D // P
    NPAIR = NH // 2
    GW = DK + 2                  # 64 data cols + rowsum-ones col + pad

    nc = bacc.Bacc("TRN2", target_bir_lowering=False, debug=False)

    qT_d = nc.dram_tensor("qT", [HTa * P, S], BF16, kind="ExternalInput").ap()
    kT_d = nc.dram_tensor("kT", [HTa * P, S], BF16, kind="ExternalInput").ap()
    vT_d = nc.dram_tensor("vT", [HTa * P, S], BF16, kind="ExternalInput").ap()
    maskT_d = nc.dram_tensor("maskT", [S, S], BF16, kind="ExternalInput").ap()
    wq_d = nc.dram_tensor("wq", [HTa * P, D], BF16, kind="ExternalInput").ap()
    wk_d = nc.dram_tensor("wk", [HTa * P, D], BF16, kind="ExternalInput").ap()
    wv_d = nc.dram_tensor("wv", [HTa * P, D], BF16, kind="ExternalInput").ap()
    wo_d = nc.dram_tensor("wo", [D, HID], BF16, kind="ExternalInput").ap()
    y_d = nc.dram_tensor("y", [HID, S], BF16, kind="ExternalOutput").ap()
    # DRAM bounce for the softmax-reciprocal partition broadcast (SBUF DMA
    # sources cannot have zero partition stride; DRAM reads can)
    rb_d = nc.dram_tensor("r_bounce", [NH * QBn, QB], BF16).ap()

    Exp = mybir.ActivationFunctionType.Exp

    with tile.TileContext(nc) as tc:
        with ExitStack() as ctx:
            persist = ctx.enter_context(tc.tile_pool(name="persist", bufs=1))
            qh_t = [persist.tile([P, S], BF16, tag=f"qh{d}", name=f"qh{d}")
                    for d in range(DC)]
            kh_t = [persist.tile([P, S], BF16, tag=f"kh{d}", name=f"kh{d}")
                    for d in range(DC)]
            vh_t = [persist.tile([P, NH * GW], BF16, tag=f"vh{s}", name=f"vh{s}")
                    for s in range(SH)]
            # per-head unnormalized x^T (rows 0..63) + rowsum (row 64), bf16
            stage_t = [persist.tile([DK + 1, S], BF16, tag=f"st{h}",
                                    name=f"st{h}") for h in range(NH)]
            xn_t = [persist.tile([P, S], BF16, tag=f"xn{p}", name=f"xn{p}")
                    for p in range(NPAIR)]
            wo_t = [persist.tile([P, HID], BF16, tag=f"wo{p}", name=f"wo{p}")
                    for p in range(NPAIR)]
            # rowsum gather rows + reciprocal scratch
            rs4 = persist.tile([NH, QB], BF16, tag="rs4", name="rs4")
            rsf = persist.tile([NH, QB], F32, tag="rsf", name="rsf")
            rff = persist.tile([NH, QB], F32, tag="rff", name="rff")
            r4b = persist.tile([NH, QB], BF16, tag="r4b", name="r4b")
            rb_t = [persist.tile([DK, QB], BF16, tag=f"rb{h}", name=f"rb{h}")
                    for h in range(NH)]

            for pr in range(NPAIR):
                nc.scalar.dma_start(wo_t[pr][:], wo_d[pr * P:(pr + 1) * P, :])

            # phase 1: projections
            with ExitStack() as ph1:
                inp = ph1.enter_context(tc.tile_pool(name="inp", bufs=1))
                wp = ph1.enter_context(tc.tile_pool(name="wp", bufs=1))
                ps1 = ph1.enter_context(
                    tc.tile_pool(name="ps1", bufs=1, space="PSUM"))

                qT_t = [inp.tile([P, S], BF16, tag=f"qT{i}", name=f"qT{i}")
                        for i in range(HTa)]
                kT_t = [inp.tile([P, S], BF16, tag=f"kT{i}", name=f"kT{i}")
                        for i in range(HTa)]
                vT_t = [inp.tile([P, S], BF16, tag=f"vT{i}", name=f"vT{i}")
                        for i in range(HTa)]
                wq_t = [wp.tile([P, D], BF16, tag=f"wq{i}", name=f"wq{i}")
                        for i in range(HTa)]
                wk_t = [wp.tile([P, D], BF16, tag=f"wk{i}", name=f"wk{i}")
                        for i in range(HTa)]
                wv_t = [wp.tile([P, D], BF16, tag=f"wv{i}", name=f"wv{i}")
                        for i in range(HTa)]
                # DMA issue cost is ~650-780ns serial per issuing engine, so
                # spread the input loads across four queues in need-order.
                for i in range(HTa):
                    sl = slice(i * P, (i + 1) * P)
                    nc.sync.dma_start(qT_t[i][:], qT_d[sl, :])
                    nc.sync.dma_start(wq_t[i][:], wq_d[sl, :])
                    nc.gpsimd.dma_start(kT_t[i][:], kT_d[sl, :])
                    nc.gpsimd.dma_start(wk_t[i][:], wk_d[sl, :])
                    nc.scalar.dma_start(vT_t[i][:], vT_d[sl, :])
                    nc.scalar.dma_start(wv_t[i][:], wv_d[sl, :])

                for (src_t, w_t, dst) in ((qT_t, wq_t, qh_t),
                                          (kT_t, wk_t, kh_t)):
                    for dc in range(DC):
                        psl = [ps1.tile([P, NS], F32, tag=f"p1_{qc}",
                                        name=f"p1_{qc}") for qc in range(NQ)]
                        for i in range(HTa):
                            for qc in range(NQ):
                                nc.tensor.matmul(
                                    psl[qc][:],
                                    w_t[i][:, dc * P:(dc + 1) * P],
                                    src_t[i][:, qc * NS:(qc + 1) * NS],
                                    start=(i == 0), stop=(i == HTa - 1))
                        for qc in range(NQ):
                            nc.vector.tensor_copy(
                                dst[dc][:, qc * NS:(qc + 1) * NS], psl[qc][:])

                # vh[k, d]: direct projection, ones cols (rowsum trick) from
                # the memset survive the grouped copy
                for sc in range(SH):
                    ps_v = ps1.tile([P, D], F32, tag="p1v", name="p1v", bufs=2)
                    for i in range(HTa):
                        nc.tensor.matmul(
                            ps_v[:],
                            vT_t[i][:, sc * P:(sc + 1) * P],
                            wv_t[i][:],
                            start=(i == 0), stop=(i == HTa - 1))
                    nc.vector.memset(vh_t[sc][:], 1.0)
                    dst_v = vh_t[sc][:].rearrange(
                        "p (h c) -> p h c", c=GW)[:, :, 0:DK]
                    src_v = ps_v[:].rearrange("p (h c) -> p h c", c=DK)
                    nc.vector.tensor_copy(dst_v, src_v)

            # phase 2+3+4 fused: attention per (q-block, head); after each
            # q-block completes, its normalize + output projection are
            # emitted interleaved into the NEXT q-block's attention as PE
            # filler work.
            with ExitStack() as ph2:
                mp = ph2.enter_context(tc.tile_pool(name="mask", bufs=1))
                pp = ph2.enter_context(tc.tile_pool(name="pexp", bufs=3))
                pmp = ph2.enter_context(tc.tile_pool(name="pmask", bufs=8))
                ysb = ph2.enter_context(tc.tile_pool(name="ysb", bufs=4))
                sps = ph2.enter_context(
                    tc.tile_pool(name="sps", bufs=3, space="PSUM"))
                xps = ph2.enter_context(
                    tc.tile_pool(name="xps", bufs=1, space="PSUM"))

                # mask tiles load on the gpsimd queue, gated behind the
                # k-projection so their 8.4MB never starves the phase-1
                # input DMAs; nothing ever sits ahead of an EXP on the
                # scalar queue.
                mask_t = [mp.tile([P, S], BF16, tag=f"m{i}", name=f"m{i}")
                          for i in range(SH)]
                gate_t = mp.tile([1, 16], BF16, tag="gate", name="gate")
                nc.gpsimd.tensor_copy(gate_t[:], kh_t[DC - 1][0:1, 0:16])
                for i in range(SH):
                    nc.gpsimd.dma_start(mask_t[i][:],
                                        maskT_d[i * P:(i + 1) * P, :])

                def attention_head(qb, h):
                    """QK -> exp -> mask -> AV (deep-lagged) for one head.

                    AV matmuls are emitted LAG chunks late so their semaphore
                    waits never block the PE."""
                    LAG = 6
                    qsl = slice(qb * QB, (qb + 1) * QB)
                    ht, hb = divmod(h, 2)
                    hsl = slice(64 * hb, 64 * hb + 64)
                    x_ps = xps.tile([P, QB], F32, tag="x", name="x")
                    pending = []

                    def emit_av(kc, pm_t):
                        for qh_ in range(QH):
                            nsl = slice(qh_ * NS, (qh_ + 1) * NS)
                            nc.tensor.matmul(
                                x_ps[:DK + 1, nsl],
                                vh_t[kc][:, h * GW:h * GW + DK + 1],
                                pm_t[:, nsl],
                                start=(kc == 0), stop=(kc == SH - 1),
                                skip_group_check=True)

                    for kc in range(SH):
                        s_ps = sps.tile([P, QB], F32, tag="s", name="s")
                        for qh_ in range(QH):
                            nsl = slice(qh_ * NS, (qh_ + 1) * NS)
                            nc.tensor.matmul(
                                s_ps[:, nsl],
                                kh_t[ht][hsl, kc * P:(kc + 1) * P],
                                qh_t[ht][hsl, qb * QB + qh_ * NS:
                                         qb * QB + (qh_ + 1) * NS],
                                start=True, stop=True)
                        p_t = pp.tile([P, QB], BF16, tag="p", name="p")
                        nc.scalar.activation(p_t[:], s_ps[:], Exp, scale=0.125)
                        pm_t = pmp.tile([P, QB], BF16, tag="pm", name="pm")
                        nc.vector.tensor_mul(pm_t[:], p_t[:], mask_t[kc][:, qsl])
                        pending.append((kc, pm_t))
                        if len(pending) > LAG:
                            emit_av(*pending.pop(0))
                    for item in pending:
                        emit_av(*item)

                    # single cast drains x rows AND the rowsum row to bf16
                    if qb == QBn - 1 and h == NH - 1:
                        # tail head: EXPs are done, ScalarE is free; emit the
                        # rowsum row first so the reciprocal chain starts
                        # while the xu columns drain
                        nc.scalar.copy(stage_t[h][DK:DK + 1, qsl],
                                       x_ps[DK:DK + 1, :])
                        nc.scalar.copy(stage_t[h][:DK, qsl], x_ps[:DK, :])
                    else:
                        nc.vector.tensor_copy(stage_t[h][:, qsl],
                                              x_ps[:DK + 1, :])

                def normalize_pair(qb, pr, tail=False):
                    """r = 1/rowsum for the pair's two heads; partition
                    broadcast via a small DRAM bounce; xn = x * r.  Emitted
                    right after the pair's second head so only the very last
                    pair's chain is ever on the critical path."""
                    qsl = slice(qb * QB, (qb + 1) * QB)
                    hs = (2 * pr, 2 * pr + 1)
                    for j, h in enumerate(hs):
                        nc.sync.dma_start(rs4[j:j + 1, :],
                                          stage_t[h][DK:DK + 1, qsl])
                    nc.vector.tensor_copy(rsf[:2, :], rs4[:2, :])
                    nc.vector.reciprocal_approx_fast(rff[:2, :], rsf[:2, :])
                    nc.vector.tensor_copy(r4b[:2, :], rff[:2, :])
                    rows = 2 * (qb * NPAIR + pr)
                    nc.sync.dma_start(rb_d[rows:rows + 2, :], r4b[:2, :])
                    for j, h in enumerate(hs):
                        eng = (nc.sync, nc.scalar if tail else nc.gpsimd)[j % 2]
                        eng.dma_start(
                            rb_t[h][:],
                            rb_d[rows + j:rows + j + 1,
                                 :].broadcast_to([DK, QB]))
                    for j, h in enumerate(hs):
                        hb = h % 2
                        hsl = slice(64 * hb, 64 * hb + 64)
                        if tail:
                            # split by q-half so the tail oproj can start
                            # after the first half lands
                            for qh_ in range(QH):
                                csl = slice(qb * QB + qh_ * NS,
                                            qb * QB + (qh_ + 1) * NS)
                                nsl = slice(qh_ * NS, (qh_ + 1) * NS)
                                nc.vector.tensor_mul(xn_t[pr][hsl, csl],
                                                     stage_t[h][:DK, csl],
                                                     rb_t[h][:, nsl])
                        else:
                            nc.vector.tensor_mul(xn_t[pr][hsl, qsl],
                                                 stage_t[h][:DK, qsl],
                                                 rb_t[h][:])

                def oproj_chunk(qb, hc, tail=False):
                    """y[hc, qb] = sum over pairs wo^T @ xn, one [P, QB]
                    PSUM chunk (one s-pool slot), staged to bf16 and written
                    out; runs as PE filler inside later q-blocks."""
                    y_ps = sps.tile([P, QB], F32, tag="s", name="yps")
                    for qh_ in range(QH):
                        nsl = slice(qh_ * NS, (qh_ + 1) * NS)
                        qc0 = qb * QH + qh_
                        for pr in range(NPAIR):
                            nc.tensor.matmul(
                                y_ps[:, nsl],
                                wo_t[pr][:, hc * P:(hc + 1) * P],
                                xn_t[pr][:, qc0 * NS:(qc0 + 1) * NS],
                                start=(pr == 0), stop=(pr == NPAIR - 1))
                    y_sb = ysb.tile([P, QB], BF16, tag="ysb", name="ysb")
                    if tail:
                        nc.scalar.copy(y_sb[:], y_ps[:])
                    else:
                        nc.vector.tensor_copy(y_sb[:], y_ps[:])
                    (nc.sync, nc.gpsimd)[hc % 2].dma_start(
                        y_d[hc * P:(hc + 1) * P, qb * QB:(qb + 1) * QB],
                        y_sb[:])

                hc_groups = [[hc for hc in range(HT) if hc % NH == h]
                             for h in range(NH)]
                for qb in range(QBn):
                    for h in range(NH):
                        attention_head(qb, h)
                        if qb >= 1:
                            for hc in hc_groups[h]:
                                oproj_chunk(qb - 1, hc)
                        if h % 2 == 1:
                            last = (qb == QBn - 1 and h == NH - 1)
                            normalize_pair(qb, h // 2, tail=last)
                for hc in range(HT):
                    oproj_chunk(QBn - 1, hc, tail=True)

    nc.compile()
    return nc


def make_in_maps(q, k, v, mask, Wq, bq, Wk, bk, Wv, bv, Wo,
                 n_cores=8, NH=4, DK=64, aug=False):
    bf = ml_dtypes.bfloat16
    B, S, HID = q.shape
    D = NH * DK
    n_hg = n_cores // B

    def with_aug(xT, bias_row):
        pad = np.zeros((P, xT.shape[1]), xT.dtype)
        pad[0, :] = bias_row
        return np.concatenate([xT, pad], axis=0)

    per_batch = {}
    for b in range(B):
        qT = np.ascontiguousarray(q[b].T).astype(bf)
        kT = np.ascontiguousarray(k[b].T).astype(bf)
        vT = np.ascontiguousarray(v[b].T).astype(bf)
        if aug:
            one = np.ones((S,), np.float32).astype(bf)
            qT, kT, vT = with_aug(qT, one), with_aug(kT, one), with_aug(vT, one)
        per_batch[b] = (qT, kT, vT,
                        np.ascontiguousarray(mask[b, 0].T != 0).astype(bf))

    in_maps = []
    for core in range(n_cores):
        b, hg = divmod(core, n_hg)
        hsl = slice(hg * D, (hg + 1) * D)
        wq = Wq[:, hsl].astype(bf)
        wk = Wk[:, hsl].astype(bf)
        wv = Wv[:, hsl].astype(bf)
        if aug:
            wq = with_aug(wq, bq[hsl].astype(bf))
            wk = with_aug(wk, bk[hsl].astype(bf))
            wv = with_aug(wv, bv[hsl].astype(bf))
        qT, kT, vT, mT = per_batch[b]
        in_maps.append(dict(
            qT=qT, kT=kT, vT=vT, maskT=mT,
            wq=np.ascontiguousarray(wq), wk=np.ascontiguousarray(wk),
            wv=np.ascontiguousarray(wv),
            wo=np.ascontiguousarray(Wo[hsl, :]).astype(bf),
        ))
    return in_maps


def combine_outputs(results, B, S, HID, bo, n_cores=8):
    n_hg = n_cores // B
    out = np.zeros((B, S, HID), np.float32)
    for core in range(n_cores):
        b = core // n_hg
        out[b] += results[core]["y"].T.astype(np.float32)
    return out + bo.astype(np.float32)


def run_mha(q, k, v, mask, Wq, bq, Wk, bk, Wv, bv, Wo, bo, trace=False):
    from concourse.bass_utils import run_bass_kernel_spmd

    B, S, HID = q.shape
    n_cores = 8
    aug = bool(np.any(bq) or np.any(bk) or np.any(bv))
    key = (S, HID, aug)
    if key not in _PROGRAM_CACHE:
        _PROGRAM_CACHE[key] = build_mha_program(S=S, HID=HID, aug=aug)
    nc = _PROGRAM_CACHE[key]
    in_maps = make_in_maps(q, k, v, mask, Wq, bq, Wk, bk, Wv, bv, Wo,
                           n_cores=n_cores, aug=aug)
    res = run_bass_kernel_spmd(nc, in_maps, list(range(n_cores)), trace=trace)
    out = combine_outputs(res.results, B, S, HID, bo, n_cores=n_cores)
    return out, res


def kernel(q, k, v, mask, Wq, bq, Wk, bk, Wv, bv, Wo, bo):
    q = np.asarray(q, np.float32)
    k = np.asarray(k, np.float32)
    v = np.asarray(v, np.float32)
    mask = np.asarray(mask)
    out, _ = run_mha(q, k, v, mask,
                     np.asarray(Wq, np.float32), np.asarray(bq, np.float32),
                     np.asarray(Wk, np.float32), np.asarray(bk, np.float32),
                     np.asarray(Wv, np.float32), np.asarray(bv, np.float32),
                     np.asarray(Wo, np.float32), np.asarray(bo, np.float32))
    return out
